# revision 1
# baseline (speedup 1.0000x reference)
"""Trainium2 Bass kernel for the DRA-C module (nn_DRA_C_30966714204439).

Sharding: data-parallel over batch B=8 across 8 NeuronCores (one image per
core); weights replicated. BatchNorm batch statistics are combined with an
in-kernel AllReduce of per-core (sum, sumsq) vectors.

Math notes (vs the jax reference):
  * patch-embed (16x16/s16 conv to 256ch) is only consumed through
    q = dl @ wq, so it is composed on the host into a single
    [65536 -> 128] projection W_eff — half the weight traffic and FLOPs.
  * The reconstruct path is 1x1-conv o nearest-upsample(16x), which
    commutes: compute y2 = (rec_w @ wo^T) @ o^T on the 14x14 grid, apply
    BN2+ReLU there (upsample preserves batch mean/var exactly since every
    small pixel is replicated 256x), and broadcast during the final
    multiply with the mask.
  * BN biases are folded into the post-allreduce scale/shift, so conv
    outputs are stored bias-free.

Precision: the two big matmuls (convm 1x1 and patch-embed) run in bf16
(inputs rounded on host); everything downstream is fp32.
"""

import numpy as np
import ml_dtypes

import concourse.bass as bass
import concourse.mybir as mybir
import concourse.tile as tile
from concourse.vector_clock import ScopedClock
from concourse.masks import make_identity

F32 = mybir.dt.float32
BF16 = mybir.dt.bfloat16
AX = mybir.AxisListType
OP = mybir.AluOpType
AF = mybir.ActivationFunctionType

N_CORES = 8
B, CD, CS, S, P, E = 8, 256, 128, 224, 16, 960
G = S // P            # 14 patches per side
NP = G * G            # 196 patches
ROWS = G * S          # 3136 pixels per row-group
N1_TOT = float(B * S * S)    # 401408 BN1 sample count
N2_TOT = float(B * NP)       # 1568  BN2 sample count
EPS = 1e-5
OUT_BF16 = True   # write output as bf16 (halves output DMA), upcast on host

# ---------------------------------------------------------------------------
# Workarounds: this container's walrus build accepts at most ONE sync-wait
# command per instruction, but Tile attaches several (tail drain waits on
# every engine; compute insts wait on multiple DMA sems). Split extras onto
# same-engine NoOps.
# ---------------------------------------------------------------------------


def _patched_drain_and_barrier(self, tick_clock, wait_clock):
    nc = self.nc
    carrier = nc.sync.nop()
    wait_clock.add_sem_waits(carrier.ins, ScopedClock({None: tick_clock.global_clock}))
    si = carrier.ins.sync_info
    waits = list(si.on_wait) if si is not None else []
    if len(waits) > 1:
        si.on_wait = waits[:1]
        for w in waits[1:]:
            extra = nc.sync.nop()
            extra.ins.sync_info = mybir.SyncInfo(on_wait=[w], on_update=[])
    nc.sync.drain()
    nc.all_engine_barrier()
    assert self.sems is not None
    popped = nc._tile_sem_poison_stack.pop()
    assert popped is self._sem_poison
    nc.clear_and_free_semaphores(list(self.sems.allocated().values()))
    nc.all_engine_barrier()


tile.TileContext._drain_and_barrier = _patched_drain_and_barrier


def _split_sync_waits(nc):
    n = 0
    for f in nc.m.functions:
        for bb in f.blocks:
            insts = list(bb.instructions)
            out = []
            changed = False
            for inst in insts:
                si = inst.sync_info
                if si is not None and len(si.on_wait) > 1:
                    waits = list(si.on_wait)
                    for w in waits[:-1]:
                        nop = mybir.InstNoOp(name=f"{inst.name}-swx{n}", ins=[], outs=[])
                        n += 1
                        nop.engine = inst.engine
                        nop.sync_info = mybir.SyncInfo(on_wait=[w], on_update=[])
                        nc.register_instruction(nop, overwrite=True)
                        out.append(nop)
                    si.on_wait = waits[-1:]
                    changed = True
                out.append(inst)
            if changed:
                bb.instructions.clear()
                for i in out:
                    bb.add_instruction(i)
    return n


# ---------------------------------------------------------------------------
# Bass program (per-core, SPMD over 8 cores)
# ---------------------------------------------------------------------------


def build_bass(reps=1):
    nc = bass.Bass(num_devices=N_CORES)

    # decoder pre-tiled on host: [i, cb, k, a, w], each (i,cb) slice contiguous
    dec = nc.dram_tensor("dec", [16, 2, 128, G, S], BF16, kind="ExternalInput")
    wconv = nc.dram_tensor("wconv", [2, 128, 128], BF16, kind="ExternalInput")
    weff = nc.dram_tensor("weff", [16, 2, 128, 16, 128], BF16, kind="ExternalInput")
    qbias = nc.dram_tensor("qbias", [128, 1], F32, kind="ExternalInput")
    transT = nc.dram_tensor("transT", [E, NP], F32, kind="ExternalInput")
    wk = nc.dram_tensor("wk", [8, 120, 128], F32, kind="ExternalInput")
    wv = nc.dram_tensor("wv", [8, 120, 128], F32, kind="ExternalInput")
    w2t = nc.dram_tensor("w2t", [128, 128], F32, kind="ExternalInput")
    bnvec = nc.dram_tensor("bnvec", [128, 6], F32, kind="ExternalInput")
    # output tile-major: [i, hf, c, a, w]; host reassembles to [c, h, w]
    out = nc.dram_tensor("out", [16, 2, CS, G // 2, S], BF16 if OUT_BF16 else F32,
                         kind="ExternalOutput")


    with tile.TileContext(nc) as tc:
        with (
            tc.tile_pool(name="const", bufs=1) as const,
            tc.tile_pool(name="decp", bufs=2) as decp,
            tc.tile_pool(name="weffp", bufs=3) as weffp,
            tc.tile_pool(name="y1p", bufs=1) as y1p,
            tc.tile_pool(name="work", bufs=3) as work,
            tc.tile_pool(name="outp", bufs=2) as outp,
            tc.tile_pool(name="small", bufs=1) as small,
            tc.tile_pool(name="psq", bufs=1, space="PSUM") as psq,
            tc.tile_pool(name="dram", bufs=1, space="DRAM") as dram,
        ):
            # ---- constants ----
            wconv_sb = const.tile([128, 2, 128], BF16)
            nc.sync.dma_start(out=wconv_sb[:], in_=wconv.rearrange("cb k m -> k cb m"))
            qbias_sb = const.tile([128, 1], F32)
            nc.sync.dma_start(out=qbias_sb[:], in_=qbias[:])
            wk_sb = const.tile([120, 8, 128], F32)
            nc.sync.dma_start(out=wk_sb[:], in_=wk.rearrange("e k m -> k e m"))
            wv_sb = const.tile([120, 8, 128], F32)
            nc.sync.dma_start(out=wv_sb[:], in_=wv.rearrange("e k m -> k e m"))
            w2t_sb = const.tile([128, 128], F32)
            nc.sync.dma_start(out=w2t_sb[:], in_=w2t[:])
            bn_sb = const.tile([128, 6], F32)
            nc.sync.dma_start(out=bn_sb[:], in_=bnvec[:])
            ident = const.tile([128, 128], F32)
            make_identity(nc, ident[:])
            ones_k = const.tile([128, 1], F32)
            nc.vector.memset(ones_k[:], 1.0)
            ones_m = const.tile([1, 128], F32)
            nc.vector.memset(ones_m[:], 1.0)
            eps1 = const.tile([1, 1], F32)
            nc.vector.memset(eps1[:], EPS)
            eps128 = const.tile([128, 1], F32)
            nc.vector.memset(eps128[:], EPS)

            # ---- phases 1-3, optionally repeated for benchmarking ----
            for rep in range(reps):
                y1_sb = y1p.tile([128, 16, ROWS], BF16, name=f"y1_{rep}", tag="y1")
                stats_sb = work.tile([128, 16, 7, 6], F32, name=f"stats_{rep}",
                                     tag="stats", bufs=1)
                psum_q = psq.tile([128, NP], F32, name=f"psq_{rep}", tag="psq")

                pcv_ctx = tc.tile_pool(name=f"pcv{rep}", bufs=1, space="PSUM")
                pcv = pcv_ctx.__enter__()

                # ---- k/v projections (overlap with decoder streaming) ----
                psum_k = pcv.tile([128, NP], F32, name=f"psum_k_{rep}", tag="pk")
                psum_v = pcv.tile([128, NP], F32, name=f"psum_v_{rep}", tag="pv")
                for e in range(8):
                    tt = work.tile([120, NP], F32, name=f"tt{e}_{rep}", tag="tt")
                    nc.sync.dma_start(out=tt[:], in_=transT[e * 120:(e + 1) * 120, :])
                    nc.tensor.matmul(psum_k[:], wk_sb[:, e, :], tt[:],
                                     start=(e == 0), stop=(e == 7))
                    nc.tensor.matmul(psum_v[:], wv_sb[:, e, :], tt[:],
                                     start=(e == 0), stop=(e == 7))
                kT_sb = small.tile([128, NP], F32, name=f"kT_{rep}", tag="kT")
                nc.scalar.copy(out=kT_sb[:], in_=psum_k[:])
                v_sb = small.tile([128, NP], F32, name=f"v_{rep}", tag="v")
                nc.scalar.copy(out=v_sb[:], in_=psum_v[:])

                for i in range(16):
                    dtile = [
                        decp.tile([128, G, S], BF16, name=f"dec{cb}_{i}_{rep}",
                                  tag=f"dec{cb}")
                        for cb in range(2)
                    ]
                    for cb in range(2):
                        nc.sync.dma_start(out=dtile[cb][:], in_=dec[i, cb])

                    # convm: 7 chunks of 448 pixels; 3 rotating psum banks,
                    # cb-accumulation back-to-back per chunk
                    flat = [dtile[cb][:].rearrange("k a w -> k (a w)")
                            for cb in range(2)]
                    for t in range(7):
                        pc = pcv.tile([128, 448], F32,
                                      name=f"pc{t}_{i}_{rep}", tag=f"pc{t % 3}")
                        for cb in range(2):
                            nc.tensor.matmul(pc[:], wconv_sb[:, cb, :],
                                             flat[cb][:, t * 448:(t + 1) * 448],
                                             start=(cb == 0), stop=(cb == 1))
                        nc.vector.bn_stats(out=stats_sb[:, i, t, :], in_=pc[:])
                        nc.scalar.copy(out=y1_sb[:, i, t * 448:(t + 1) * 448],
                                       in_=pc[:])

                    # patch-embed -> q accumulation
                    for cb in range(2):
                        wt = weffp.tile([128, 16, 128], BF16, name=f"we{i}_{cb}_{rep}", tag="we")
                        nc.sync.dma_start(out=wt[:], in_=weff[i, cb])
                        dj = dtile[cb][:].rearrange("k a (q j) -> k j a q", j=16)
                        for j in range(16):
                            nc.tensor.matmul(
                                psum_q[:], wt[:, j, :], dj[:, j, :, :],
                                start=(i == 0 and cb == 0 and j == 0),
                                stop=(i == 15 and cb == 1 and j == 15),
                            )

                pcv_ctx.__exit__(None, None, None)
                ps2_ctx = tc.tile_pool(name=f"ps2_{rep}", bufs=1, space="PSUM")
                ps2 = ps2_ctx.__enter__()

                # ---- per-core BN1 stats -> (sum, sumsq); AllReduce #1
                # launches here and overlaps the attention chain below ----
                mv1 = small.tile([128, 2], F32, name=f"mv1_{rep}", tag="mv1")
                nc.vector.bn_aggr(out=mv1[:], in_=stats_sb[:])
                red = small.tile([128, 4], F32, name=f"red_{rep}", tag="red")
                t0 = small.tile([128, 1], F32, name=f"t0_{rep}", tag="t0")
                nc.vector.tensor_mul(t0[:], mv1[:, 0:1], mv1[:, 0:1])
                nc.vector.tensor_add(red[:, 1:2], mv1[:, 1:2], t0[:])
                nc.scalar.mul(out=red[:, 1:2], in_=red[:, 1:2], mul=float(S * S))
                nc.scalar.mul(out=red[:, 0:1], in_=mv1[:, 0:1], mul=float(S * S))

                # ---- attention tail ----
                q_sb = small.tile([128, NP], F32, name=f"q_{rep}", tag="q")
                nc.vector.tensor_scalar_add(q_sb[:], psum_q[:], qbias_sb[:])

                # transposes: q[c,n] -> qT chunks [n,c];  kT[d,n] -> k chunks [n,d]
                qT = []
                kc = []
                for h in range(2):
                    pt = ps2.tile([98, 128], F32, name=f"ptq{h}_{rep}", tag="px", bufs=2)
                    nc.tensor.transpose(pt[:], q_sb[:, h * 98:(h + 1) * 98], ident[:])
                    sb = small.tile([98, 128], F32, name=f"qT{h}_{rep}", tag=f"qT{h}")
                    nc.scalar.copy(out=sb[:], in_=pt[:])
                    qT.append(sb)
                    pt2 = ps2.tile([98, 128], F32, name=f"ptk{h}_{rep}", tag="px", bufs=2)
                    nc.tensor.transpose(pt2[:], kT_sb[:, h * 98:(h + 1) * 98], ident[:])
                    sb2 = small.tile([98, 128], F32, name=f"kc{h}_{rep}", tag=f"kc{h}")
                    nc.scalar.copy(out=sb2[:], in_=pt2[:])
                    kc.append(sb2)

                psum_sim = ps2.tile([128, 128], F32, name=f"psum_sim_{rep}", tag="px", bufs=2)
                for h in range(2):
                    nc.tensor.matmul(psum_sim[:], qT[h][:], kc[h][:],
                                     start=(h == 0), stop=(h == 1))

                # instance norm over the whole 128x128 map, computed on PSUM;
                # normalize+exp fuse into one activation:
                #   softmax((x-m)*r) rows: exp(x*r - m*r) / rowsum
                rs2 = small.tile([128, 2], F32, name=f"rs2_{rep}", tag="rs2")
                nc.vector.tensor_reduce(out=rs2[:, 0:1], in_=psum_sim[:], axis=AX.X, op=OP.add)
                scr = small.tile([128, 128], F32, name=f"scr_{rep}", tag="scr")
                nc.scalar.square(out=scr[:], in_=psum_sim[:])
                nc.vector.tensor_reduce(out=rs2[:, 1:2], in_=scr[:], axis=AX.X, op=OP.add)
                ptot = ps2.tile([1, 2], F32, name=f"ptot_{rep}", tag="px", bufs=2)
                nc.tensor.matmul(ptot[:], ones_k[:], rs2[:], start=True, stop=True)
                tot = small.tile([1, 2], F32, name=f"tot_{rep}", tag="tot")
                nc.scalar.mul(out=tot[:], in_=ptot[:], mul=1.0 / 16384.0)
                m2 = small.tile([1, 1], F32, name=f"m2_{rep}", tag="m2")
                nc.vector.tensor_mul(m2[:], tot[:, 0:1], tot[:, 0:1])
                var_i = small.tile([1, 1], F32, name=f"vari_{rep}", tag="vari")
                nc.vector.tensor_sub(var_i[:], tot[:, 1:2], m2[:])
                sd_i = small.tile([1, 1], F32, name=f"sdi_{rep}", tag="sdi")
                nc.scalar.activation(out=sd_i[:], in_=var_i[:], func=AF.Sqrt, bias=eps1[:])
                mr = small.tile([1, 2], F32, name=f"mr_{rep}", tag="mr")
                nc.vector.reciprocal(out=mr[:, 1:2], in_=sd_i[:])
                # mr[0] = -mean * rstd  (the Exp bias)
                nc.vector.tensor_mul(mr[:, 0:1], tot[:, 0:1], mr[:, 1:2])
                nc.vector.tensor_scalar_mul(mr[:, 0:1], mr[:, 0:1], -1.0)
                pbc = ps2.tile([128, 2], F32, name=f"pbc_{rep}", tag="px", bufs=2)
                nc.tensor.matmul(pbc[:], ones_m[:], mr[:], start=True, stop=True)
                bc = small.tile([128, 2], F32, name=f"bc_{rep}", tag="bc")
                nc.scalar.copy(out=bc[:], in_=pbc[:])
                nc.scalar.activation(out=scr[:], in_=psum_sim[:], func=AF.Exp,
                                     bias=bc[:, 0:1], scale=bc[:, 1:2])
                ssum = small.tile([128, 1], F32, name=f"ssum_{rep}", tag="ssum")
                nc.vector.tensor_reduce(out=ssum[:], in_=scr[:], axis=AX.X, op=OP.add)
                rinv = small.tile([128, 1], F32, name=f"rinv_{rep}", tag="rinv")
                nc.vector.reciprocal(out=rinv[:], in_=ssum[:])
                nc.vector.tensor_scalar_mul(scr[:], scr[:], rinv[:])

                # oT = sim @ v ; y2 = W2 @ oT
                pst = ps2.tile([128, 128], F32, name=f"pst_{rep}", tag="px", bufs=2)
                nc.tensor.transpose(pst[:], scr[:], ident[:])
                simT = small.tile([128, 128], F32, name=f"simT_{rep}", tag="simT")
                nc.scalar.copy(out=simT[:], in_=pst[:])
                psum_o = ps2.tile([128, NP], F32, name=f"psum_o_{rep}", tag="px", bufs=2)
                nc.tensor.matmul(psum_o[:], simT[:], v_sb[:], start=True, stop=True)
                oT_sb = small.tile([128, NP], F32, name=f"oT_{rep}", tag="oT")
                nc.scalar.copy(out=oT_sb[:], in_=psum_o[:])
                psum_y2 = ps2.tile([128, NP], F32, name=f"psum_y2_{rep}",
                                   tag="py2", bufs=1)
                nc.tensor.matmul(psum_y2[:], w2t_sb[:], oT_sb[:], start=True, stop=True)
                y2_sb = psum_y2

                # per-core BN2 partial sums; AllReduce #2 (overlaps the
                # mask-relu passes, which only depend on AllReduce #1)
                nc.vector.tensor_reduce(out=red[:, 2:3], in_=y2_sb[:], axis=AX.X, op=OP.add)
                scr2 = small.tile([128, NP], F32, name=f"scr2_{rep}", tag="scr2")
                nc.scalar.square(out=scr2[:], in_=y2_sb[:])
                nc.vector.tensor_reduce(out=red[:, 3:4], in_=scr2[:], axis=AX.X, op=OP.add)
                cc_in = dram.tile([128, 4], F32, name=f"cci_{rep}", tag="cci")
                cc_out = dram.tile([128, 4], F32, name=f"cco_{rep}", tag="cco",
                                   addr_space="Shared")
                nc.sync.dma_start(out=cc_in[:], in_=red[:])
                nc.gpsimd.collective_compute(
                    "AllReduce", OP.add, replica_groups=[list(range(N_CORES))],
                    ins=[cc_in[:]], outs=[cc_out[:]])
                glob = small.tile([128, 4], F32, name=f"glob_{rep}", tag="glob")
                nc.sync.dma_start(out=glob[:], in_=cc_out[:])

                # ---- BN affine folds: scale/shift per channel ----
                # bnvec columns: 0 bn1_g, 1 bn1_b, 2 convm_b, 3 bn2_g, 4 bn2_b, 5 rec_b
                def bn_fold(gl, n_tot, g_col, b_col, cbias_col, tagn):
                    mean = small.tile([128, 1], F32, name=f"mean{tagn}_{rep}", tag=f"mean{tagn}")
                    nc.scalar.mul(out=mean[:], in_=gl[:, 0:1], mul=1.0 / n_tot)
                    ex2 = small.tile([128, 1], F32, name=f"ex2{tagn}_{rep}", tag=f"ex2{tagn}")
                    nc.scalar.mul(out=ex2[:], in_=gl[:, 1:2], mul=1.0 / n_tot)
                    msq = small.tile([128, 1], F32, name=f"msq{tagn}_{rep}", tag=f"msq{tagn}")
                    nc.vector.tensor_mul(msq[:], mean[:], mean[:])
                    var = small.tile([128, 1], F32, name=f"var{tagn}_{rep}", tag=f"var{tagn}")
                    nc.vector.tensor_sub(var[:], ex2[:], msq[:])
                    sd = small.tile([128, 1], F32, name=f"sd{tagn}_{rep}", tag=f"sd{tagn}")
                    nc.scalar.activation(out=sd[:], in_=var[:], func=AF.Sqrt,
                                         bias=eps128[:])
                    rstd = small.tile([128, 1], F32, name=f"rstd{tagn}_{rep}", tag=f"rstd{tagn}")
                    nc.vector.reciprocal(out=rstd[:], in_=sd[:])
                    scale = small.tile([128, 1], F32, name=f"scale{tagn}_{rep}", tag=f"scale{tagn}")
                    nc.vector.tensor_mul(scale[:], bn_sb[:, g_col:g_col + 1], rstd[:])
                    mt = small.tile([128, 1], F32, name=f"mt{tagn}_{rep}", tag=f"mt{tagn}")
                    nc.vector.tensor_add(mt[:], mean[:], bn_sb[:, cbias_col:cbias_col + 1])
                    ms = small.tile([128, 1], F32, name=f"ms{tagn}_{rep}", tag=f"ms{tagn}")
                    nc.vector.tensor_mul(ms[:], mt[:], scale[:])
                    shift = small.tile([128, 1], F32, name=f"shift{tagn}_{rep}", tag=f"shift{tagn}")
                    nc.vector.tensor_sub(shift[:], bn_sb[:, b_col:b_col + 1], ms[:])
                    return scale, shift

                scale1, shift1 = bn_fold(glob[:, 0:2], N1_TOT, 0, 1, 2, "1")
                scale2, shift2 = bn_fold(glob[:, 2:4], N2_TOT, 3, 4, 5, "2")

                z_sb = small.tile([128, NP], F32, name=f"z_{rep}", tag="z")
                nc.scalar.activation(out=z_sb[:], in_=y2_sb[:], func=AF.Relu,
                                     bias=shift2[:], scale=scale2[:])

                ps2_ctx.__exit__(None, None, None)

                # ---- phase 3: mask apply + upsample-multiply + store ----
                # split each row-group in half and run with 4 buffers so the
                # ACT(relu) -> DVE(mul) -> DMA(store) chain pipelines.
                G2 = G // 2
                zp = z_sb[:].ap[0]  # [step, count] of the partition dim
                for i in range(16):
                    for hf in range(2):
                        a0 = hf * G2
                        ot = outp.tile([128, G2, G, 16],
                                       BF16 if OUT_BF16 else F32,
                                       name=f"ot{i}_{hf}_{rep}", tag="ot", bufs=6)
                        ysl = y1_sb[:, i, a0 * S:(a0 + G2) * S]
                        nc.scalar.activation(
                            out=ot[:],
                            in_=ysl.rearrange("c (a q j) -> c a q j", a=G2, j=16),
                            func=AF.Relu, bias=shift1[:], scale=scale1[:])
                        zs = z_sb[:, a0 * G:(a0 + G2) * G]
                        zbc = bass.AP(
                            tensor=z_sb.tensor, offset=zs.offset,
                            ap=[list(zp), [G, G2], [1, G], [0, 16]])
                        nc.vector.tensor_mul(ot[:], ot[:], zbc)
                        nc.sync.dma_start(
                            out=out[i, hf],
                            in_=ot[:].rearrange("c a q j -> c a (q j)"))


    return nc


# ---------------------------------------------------------------------------
# Host wrapper
# ---------------------------------------------------------------------------

_CACHE = {}


def _prep_shared(pe_w, pe_b, convm_w, wq, wk, wv, wo, rec_w,
                 bn1_g, bn1_b, convm_b, bn2_g, bn2_b, rec_b):
    bf = ml_dtypes.bfloat16
    # W_eff[cin, i, j, cs] = sum_co pe_w[co, cin, i, j] * wq[co, cs]
    weff = (pe_w.reshape(CD, CD * P * P).T @ wq).reshape(CD, P, P, CS)
    # arrange [i, cb, k, j, m]
    weff_a = np.ascontiguousarray(
        weff.reshape(2, 128, P, P, CS).transpose(2, 0, 1, 3, 4)).astype(bf)
    qbias = np.ascontiguousarray((pe_b @ wq).reshape(CS, 1)).astype(np.float32)
    wc = convm_w[:, :, 0, 0]                     # [cs, cd]
    wconv_a = np.ascontiguousarray(wc.T.reshape(2, 128, CS)).astype(bf)
    wk_a = np.ascontiguousarray(wk.reshape(8, 120, CS)).astype(np.float32)
    wv_a = np.ascontiguousarray(wv.reshape(8, 120, CS)).astype(np.float32)
    rec_mat = rec_w[:, :, 0, 0]                  # [cs_out, c2]
    w2t = np.ascontiguousarray(wo @ rec_mat.T).astype(np.float32)   # [c, cs]
    bnvec = np.ascontiguousarray(
        np.stack([bn1_g, bn1_b, convm_b, bn2_g, bn2_b, rec_b], axis=1)
    ).astype(np.float32)
    return dict(weff=weff_a, qbias=qbias, wconv=wconv_a, wk=wk_a, wv=wv_a,
                w2t=w2t, bnvec=bnvec)


def make_in_maps(decoder, trans, pe_w, pe_b, convm_w, convm_b, bn1_g, bn1_b,
                 wq, wk, wv, wo, rec_w, rec_b, bn2_g, bn2_b):
    bf = ml_dtypes.bfloat16
    shared = _prep_shared(pe_w, pe_b, convm_w, wq, wk, wv, wo, rec_w,
                          bn1_g, bn1_b, convm_b, bn2_g, bn2_b, rec_b)
    dec_bf = np.asarray(decoder).astype(bf)
    # [c, h, w] -> [i, cb, k, a, w] with h = a*16 + i, c = cb*128 + k
    dec_t = np.ascontiguousarray(
        dec_bf.reshape(B, 2, 128, G, P, S).transpose(0, 4, 1, 2, 3, 5))
    in_maps = []
    for b in range(B):
        m = dict(shared)
        m["dec"] = dec_t[b]
        m["transT"] = np.ascontiguousarray(np.asarray(trans[b]).T).astype(np.float32)
        in_maps.append(m)
    return in_maps


def get_nc(reps=1):
    key = f"nc{reps}"
    if key not in _CACHE:
        nc = build_bass(reps)
        _split_sync_waits(nc)
        _CACHE[key] = nc
    return _CACHE[key]


def unshard_out(raw):
    # [i, hf, c, a, w] -> [c, h, w], h = (hf*7 + a)*16 + i
    return np.ascontiguousarray(
        raw.transpose(2, 1, 3, 0, 4).reshape(CS, S, S)).astype(np.float32)


def kernel(**inputs):
    from concourse.bass_utils import run_bass_kernel_spmd

    inputs = {k: np.asarray(v) for k, v in inputs.items()}
    in_maps = make_in_maps(**inputs)
    nc = get_nc()
    res = run_bass_kernel_spmd(nc, in_maps, core_ids=list(range(N_CORES)))
    return np.stack([unshard_out(res.results[b]["out"]) for b in range(B)], axis=0)

